# revision 22
# baseline (speedup 1.0000x reference)
"""Multi-head attention Bass/Tile kernel for Trainium2 (8 NeuronCores), v2.

Problem: B=32, NQ=NK=512, IN_DIM=512, H=8 heads, E=64, OUT_DIM=512, fp32.
Sharding: data-parallel over batch — 8 cores x 4 batches, all 8 heads local
per core, so no collectives are needed.

v2 changes over the 147us baseline (all trace-driven):
  - Q/K projections run in fp8e4 DoubleRow (contraction 256 rows/matmul):
    halves their PE streaming time. Weights are pre-scaled x64 so they sit
    in fp8's normal range; the 1/4096 compensation is folded into the exp
    scale. V / scores / AV / out-proj stay bf16 (fp8 there fails accuracy
    or was measured slower). Host packs all fp8 operands.
  - Software pipeline: batch b+1's Q/K projection matmuls are interleaved
    into batch b's per-head AV/score slots. The exp ACT engine (1.2 GHz,
    2.3us/head) otherwise paces the PE (1.7us/head) via the score-PSUM
    rotation; the extra ~0.9us of projection work per slot gives ACT the
    slack it needs.
  - p-major host layouts (partition-major, chunks along the free dim) make
    every DMA row 2-4KB contiguous -> fewer, larger descriptors (the HWDGE
    is ~180ns/descriptor), and one DMA instruction per tensor per batch.
  - Vaug's softmax-denominator pad columns ((1-mask) replicated 8x64) come
    straight from a host-precomputed DRAM tensor instead of DVE broadcast
    copies (-3.4us/batch on Vector).
  - Output is staged p-major so the unshard is a host-side transpose and
    the final DMAs are 2KB-row transfers split across both HWDGE queues.

Dataflow per (core, batch) (see baseline docstring for the Vaug/denominator
trick): scoresT = KT_h.T @ QT_h per k-tile; exT = exp(ESC*scoresT); AV
lhsT = [pad(1-mask) | V_h] so PSUM partitions 0:64 hold the softmax
denominator; fast-reciprocal + multiply normalize; out-proj accumulates
4 head-pair chunks.
"""

import os
import sys
import types

sys.path.insert(0, "/opt/trn_rl_repo")

import numpy as np

B, NQ, NK, DIN, H, E, DOUT = 32, 512, 512, 512, 8, 64, 512
NCORES = 8
BPC = B // NCORES   # batches per core
P = 128
C = DIN // P        # bf16 contraction chunks (V path)
C2 = 2              # fp8 DoubleRow contraction chunks (256 rows each)
T = NK // P         # k tiles
G = H // 2          # head-pair groups
HE = H * E          # 512
SW = 64.0           # fp8 weight pre-scale
ESC = 0.125 / (SW * SW)  # exp scale compensating the x64 wq/wk pre-scales

_CACHE = {}
LAST_RESULT = None


def _install_ntff_hook():
    """trn_boot can't register the NTFF profile hook (antenv stub lacks
    axon_hooks); recreate the module so BASS_TRACE=1 profiling works."""
    if "antenv.axon_hooks" in sys.modules:
        return
    try:
        import antenv

        mod = types.ModuleType("antenv.axon_hooks")
        holder = [None]
        mod.set_axon_ntff_profile_hook = lambda h: holder.__setitem__(0, h)
        mod.get_axon_ntff_profile_hook = lambda: holder[0]
        sys.modules["antenv.axon_hooks"] = mod
        antenv.axon_hooks = mod
        if "/root/.axon_site" not in sys.path:
            sys.path.append("/root/.axon_site")
        from trn_agent_boot.trn_boot import _ntff_profile_via_ctypes

        mod.set_axon_ntff_profile_hook(
            _ntff_profile_via_ctypes("/opt/axon/libaxon_pjrt.so")
        )
    except Exception:
        pass


def _build(fp8_qk=True):
    import concourse.bass as bass  # noqa: F401
    import concourse.mybir as mybir
    import concourse.tile as tile
    from concourse import bacc

    f32 = mybir.dt.float32
    f8 = mybir.dt.float8e4
    bf16 = mybir.dt.bfloat16
    DR = mybir.MatmulPerfMode.DoubleRow

    nc = bacc.Bacc("TRN2", target_bir_lowering=False, debug=False,
                   num_devices=NCORES)

    # p-major DRAM layouts: partition index first, all chunks along the free
    # dim, so each DMA row is one long contiguous block. All dram tensors
    # are kept <= 3-D (higher ranks break the PJRT compile hook); device
    # code views them via rearrange.
    if fp8_qk:
        qTi = nc.dram_tensor("qTi", [BPC, P, 2 * C2 * NQ], f8,
                             kind="ExternalInput")
        kTi = nc.dram_tensor("kTi", [BPC, P, 2 * C2 * NK], f8,
                             kind="ExternalInput")
        # fused (weights | batch-0 inputs) per DoubleRow chunk
        wqf = nc.dram_tensor("wqf", [C2, P, 2 * (HE + NQ)], f8,
                             kind="ExternalInput")
        wkf = nc.dram_tensor("wkf", [C2, P, 2 * (HE + NK)], f8,
                             kind="ExternalInput")
    else:
        qTi = nc.dram_tensor("qTi", [BPC, P, C * NQ], bf16,
                             kind="ExternalInput")
        kTi = nc.dram_tensor("kTi", [BPC, P, C * NK], bf16,
                             kind="ExternalInput")
        wqf = nc.dram_tensor("wqf", [C, P, HE + NQ], bf16,
                             kind="ExternalInput")
        wkf = nc.dram_tensor("wkf", [C, P, HE + NK], bf16,
                             kind="ExternalInput")
    vT = nc.dram_tensor("vT", [BPC, P, C * NK], bf16, kind="ExternalInput")
    wv = nc.dram_tensor("wv", [P, C * HE], bf16, kind="ExternalInput")
    wo = nc.dram_tensor("wo", [P, C * DOUT], bf16, kind="ExternalInput")
    mrep = nc.dram_tensor("mrep", [BPC, P, T * NQ], bf16,
                          kind="ExternalInput")
    mnot = nc.dram_tensor("mnot", [BPC, P, T], f32, kind="ExternalInput")
    out = nc.dram_tensor("out", [BPC, P, (NQ // P) * DOUT], bf16,
                         kind="ExternalOutput")

    NWQ = C2 if fp8_qk else C

    with tile.TileContext(nc) as tc:
        with (
            tc.tile_pool(name="consts", bufs=1) as cpool,
            tc.tile_pool(name="io", bufs=3) as iopool,
            tc.tile_pool(name="work", bufs=2) as wpool,
            tc.tile_pool(name="expool", bufs=7) as expool,
            tc.tile_pool(name="ps_sc", bufs=3, space="PSUM") as ps_sc,
            tc.tile_pool(name="ps_acc", bufs=2, space="PSUM") as ps_acc,
        ):
            if fp8_qk:
                wq_sbs = [cpool.tile([P, 2, HE + NQ], f8, tag=f"wq{c}", name=f"wq{c}")
                          for c in range(C2)]
                wk_sbs = [cpool.tile([P, 2, HE + NK], f8, tag=f"wk{c}", name=f"wk{c}")
                          for c in range(C2)]
            else:
                wq_sbs = [cpool.tile([P, HE + NQ], bf16, tag=f"wq{c}", name=f"wq{c}")
                          for c in range(C)]
                wk_sbs = [cpool.tile([P, HE + NK], bf16, tag=f"wk{c}", name=f"wk{c}")
                          for c in range(C)]
            wv_sb = cpool.tile([P, C, HE], bf16, tag="wv")
            wo_sb = cpool.tile([P, C, DOUT], bf16, tag="wo")
            warm = cpool.tile([P, NQ], bf16, tag="warm")
            nc.vector.memset(warm[:], 0.0)

            # rolling per-batch tiles (allocated one batch ahead)
            qT_sb, kT_sb, vT_sbs, mn_sbs = {}, {}, {}, {}
            Vaug, QTs, KTs = {}, {}, {}

            mrep_sbs = {}

            def alloc_batch(b):
                """Allocate batch-b tiles and issue its input DMAs. Called
                one batch ahead so transfers fully overlap compute."""
                # head-h AV lhsT block = Vaug[:, t, h, :, :] = [pad64|V64],
                # contiguous (the PE stationary AP allows only 1 free dim)
                Vaug[b] = wpool.tile([P, T, H, 2, E], bf16, tag="Va",
                                     name=f"Va{b}")
                mrep_sbs[b] = iopool.tile([P, T, NQ], bf16, tag="mrep",
                                          name=f"mrep{b}")
                QTs[b] = wpool.tile([P, G, NQ], bf16, tag="QT", name=f"QT{b}")
                KTs[b] = wpool.tile([P, G, NK], bf16, tag="KT", name=f"KT{b}")
                vT_sbs[b] = iopool.tile([P, C, NK], bf16, tag="vT",
                                        name=f"vT{b}")
                mn_sbs[b] = iopool.tile([P, T], f32, tag="mn", name=f"mn{b}")
                if b == 0:
                    for c in range(NWQ):
                        nc.sync.dma_start(wq_sbs[c][:], wqf[c])
                    for c in range(NWQ):
                        nc.scalar.dma_start(wk_sbs[c][:], wkf[c])
                    nc.scalar.dma_start(wv_sb[:], wv[:])
                    nc.scalar.dma_start(wo_sb[:], wo[:])
                else:
                    qT_sb[b] = iopool.tile(
                        [P, 2, C2, NQ] if fp8_qk else [P, C, NQ],
                        f8 if fp8_qk else bf16, tag="qT", name=f"qT{b}")
                    kT_sb[b] = iopool.tile(
                        [P, 2, C2, NK] if fp8_qk else [P, C, NK],
                        f8 if fp8_qk else bf16, tag="kT", name=f"kT{b}")
                    nc.sync.dma_start(qT_sb[b][:], qTi[b])
                    nc.sync.dma_start(kT_sb[b][:], kTi[b])
                nc.sync.dma_start(vT_sbs[b][:], vT[b])
                # denominator pad columns ((1-mask) replicated 8Hx64E) come
                # pre-replicated from DRAM; one linear-read DVE copy scatters
                # them into Vaug's pad blocks at the start of batch b.
                nc.sync.dma_start(mrep_sbs[b][:], mrep[b])
                nc.sync.dma_start(mn_sbs[b][:], mnot[b])

            def proj_unit(bn, i):
                """One Q or K projection group for batch bn: i=0..7 maps to
                (Q,g0),(K,g0),(Q,g1),(K,g1),... Emits the matmuls + the
                PSUM->SBUF bf16 cast."""
                is_q, g = (i % 2 == 0), i // 2
                w_sbs = wq_sbs if is_q else wk_sbs
                pj = ps_acc.tile([P, NQ], f32, tag="acc", name="pj")
                if fp8_qk:
                    for cc in range(C2):
                        if bn == 0:
                            rhs = w_sbs[cc][:, :, HE:]
                        else:
                            x = qT_sb[bn] if is_q else kT_sb[bn]
                            rhs = x[:, :, cc, :]
                        nc.tensor.matmul(
                            pj[:], lhsT=w_sbs[cc][:, :, g * P:(g + 1) * P],
                            rhs=rhs, start=(cc == 0), stop=(cc == C2 - 1),
                            perf_mode=DR)
                else:
                    for c in range(C):
                        if bn == 0:
                            rhs = w_sbs[c][:, HE:]
                        else:
                            x = qT_sb[bn] if is_q else kT_sb[bn]
                            rhs = x[:, c, :]
                        nc.tensor.matmul(
                            pj[:], lhsT=w_sbs[c][:, g * P:(g + 1) * P],
                            rhs=rhs, start=(c == 0), stop=(c == C - 1))
                dst = QTs[bn] if is_q else KTs[bn]
                nc.vector.tensor_copy(out=dst[:, g, :], in_=pj[:])

            def emit_scores_exp(b, h):
                g, hh = h // 2, h % 2
                es = slice(hh * E, (hh + 1) * E)
                exT = expool.tile([P, T, NQ], bf16, name=f"exT{b}_{h}",
                                  tag="ex")
                sc0 = ps_sc.tile([P, 2 * NQ], f32, name="sc0", tag="sc")
                for t in range(2):
                    nc.tensor.matmul(
                        sc0[:, t * NQ:(t + 1) * NQ],
                        lhsT=KTs[b][es, g, t * P:(t + 1) * P],
                        rhs=QTs[b][es, g, :], start=True, stop=True)
                nc.scalar.activation(
                    exT[:, 0:2, :], sc0[:].rearrange("p (t n) -> p t n", t=2),
                    mybir.ActivationFunctionType.Exp,
                    scale=ESC if fp8_qk else 0.125)
                sc1 = ps_sc.tile([P, 2 * NQ], f32, name="sc1", tag="sc")
                for t in range(2, T):
                    nc.tensor.matmul(
                        sc1[:, (t - 2) * NQ:(t - 1) * NQ],
                        lhsT=KTs[b][es, g, t * P:(t + 1) * P],
                        rhs=QTs[b][es, g, :], start=True, stop=True)
                nc.scalar.activation(
                    exT[:, 2:4, :], sc1[:].rearrange("p (t n) -> p t n", t=2),
                    mybir.ActivationFunctionType.Exp,
                    scale=ESC if fp8_qk else 0.125)
                return exT

            from concourse.dve_ops import (
                RECIP_APPROX_FAST_CONSTS as _rc,
                RECIPROCAL_APPROX_FAST as _rf,
            )

            def rcp_mul(b, h, Unorm, up):
                """Normalize head h: up[0:64] holds 64 identical denominator
                copies; fast reciprocal + multiply into Unorm. (A fused
                single-op version is impossible: the DVE can read only one
                non-scalar PSUM operand and both den and U live in PSUM.)"""
                g, hh = h // 2, h % 2
                es = slice(hh * E, (hh + 1) * E)
                rcp = wpool.tile([E, NQ], bf16, tag="rcp")
                nc.vector._custom_dve(_rf, out=rcp[:], in0=up[0:E, :],
                                      s0=_rc["s0"], s1=_rc["s1"],
                                      imm2=_rc["imm2"])
                nc.vector.tensor_mul(out=Unorm[es, g, :],
                                     in0=up[E:2 * E, :], in1=rcp[:])

            def v_proj_half(b, half):
                """V projection half (2 k-tiles) -> Vaug V blocks with masked
                rows zeroed."""
                pv = ps_sc.tile([P, 2 * HE], f32, tag="sc",
                                name=f"pv{b}_{half}")
                for tt in range(2):
                    t = 2 * half + tt
                    for c in range(C):
                        nc.tensor.matmul(
                            pv[:, tt * HE:(tt + 1) * HE],
                            lhsT=vT_sbs[b][:, c, t * P:(t + 1) * P],
                            rhs=wv_sb[:, c, :],
                            start=(c == 0), stop=(c == C - 1))
                for tt in range(2):
                    t = 2 * half + tt
                    nc.vector.tensor_scalar_mul(
                        Vaug[b][:, t, :, 1, :],
                        pv[:, tt * HE:(tt + 1) * HE].rearrange(
                            "p (h e) -> p h e", e=E),
                        mn_sbs[b][:, t:t + 1])

            def pad_scatter(b):
                # scatter the pre-replicated (1-mask) pad blocks into Vaug
                nc.vector.tensor_copy(
                    out=Vaug[b][:, :, :, 0, :],
                    in_=mrep_sbs[b][:].rearrange("p t (h e) -> p t h e", e=E))

            # PE warmup: dummy matmuls bridge the idle window while the first
            # input DMAs land so the DVFS p-state ramp (max clock after ~3us
            # of continuous execution) completes before the real projections.
            alloc_batch(0)
            wps = ps_acc.tile([P, NQ], f32, tag="acc", name="wps")
            for _ in range(9):
                nc.tensor.matmul(wps[:], lhsT=warm[:, 0:P], rhs=warm[:],
                                 start=True, stop=True)

            exTs = {}    # b -> exT tiles emitted so far
            Unorms = {}
            # which (Q,K) projection group pairs of b+1 run in slot h of b
            unit_slots = {0: (0, 1), 1: (2, 3), 2: (4, 5), 4: (6, 7)}

            for b in range(BPC):
                if b + 1 < BPC:
                    alloc_batch(b + 1)
                Unorms[b] = wpool.tile([P, G, NQ], bf16, tag="Un",
                                       name=f"Un{b}")
                Unorm = Unorms[b]
                ob = iopool.tile([P, NQ // P, DOUT], bf16, tag="ob",
                                 name=f"ob{b}")

                if b == 0:
                    # batch-0 startup runs inline (later batches have all of
                    # this pre-issued from the previous batch's slots).
                    # Q/K projections are DMA-paced; g0 first so the first
                    # scores can start as early as possible.
                    for i in range(8):
                        proj_unit(0, i)
                    # LDWEIGHTS filler keeps the PE clock ramped through the
                    # chunk-arrival gaps (no PSUM side effects).
                    for _ in range(8):
                        nc.tensor.ldweights(warm[:, 0:P])
                    pad_scatter(0)
                    exTs[0] = [emit_scores_exp(0, 0), emit_scores_exp(0, 1)]
                    v_proj_half(0, 0)
                    v_proj_half(0, 1)
                    # first AV waits ~0.7us for the DVE mask-apply
                    for _ in range(6):
                        nc.tensor.ldweights(warm[:, 0:P])

                # ---- per-head attention. Scores run 2 heads ahead of AV.
                # ALL of batch b+1's pre-work (Q/K projections, first two
                # score+exp blocks, V projection, pad scatter) is spread
                # across b's slots so the exp ACT queue never drains at the
                # batch boundary and the PE always has slack work.
                nxt = b + 1 if b + 1 < BPC else None
                for h in range(H):
                    exT = exTs[b][h]
                    up = ps_acc.tile([P, NQ], f32, tag="acc", name="up")
                    for t in range(T):
                        nc.tensor.matmul(
                            up[:],
                            lhsT=Vaug[b][:, t, h, :, :],
                            rhs=exT[:, t, :],
                            start=(t == 0), stop=(t == T - 1))

                    # normalize first: it is the AV-PSUM release on the DVE
                    # queue, so it must not sit behind the proj cast
                    rcp_mul(b, h, Unorm, up)
                    if nxt is not None and h in unit_slots:
                        for i in unit_slots[h]:
                            proj_unit(nxt, i)
                    if h + 2 < H:
                        exTs[b].append(emit_scores_exp(b, h + 2))
                    if nxt is not None:
                        if h == 5:
                            exTs[nxt] = [emit_scores_exp(nxt, 0)]
                        elif h == 6:
                            exTs[nxt].append(emit_scores_exp(nxt, 1))
                            v_proj_half(nxt, 0)
                        elif h == 7:
                            v_proj_half(nxt, 1)
                            pad_scatter(nxt)

                    # Last batch: nothing overlaps the final out-projection,
                    # so emit head-pair chunks 0..2 between AV h6 and h7.
                    if b == BPC - 1 and h == H - 2:
                        po2s = [ps_sc.tile([P, 2 * DOUT], f32, tag="sc",
                                           name=f"po2{i}") for i in range(2)]
                        for c in range(C - 1):
                            for pair in range(2):
                                for j in range(2):
                                    qt = 2 * pair + j
                                    nc.tensor.matmul(
                                        po2s[pair][:, j * DOUT:(j + 1) * DOUT],
                                        lhsT=Unorm[:, c, qt * P:(qt + 1) * P],
                                        rhs=wo_sb[:, c, :],
                                        start=(c == 0), stop=False,
                                        skip_group_check=True)

                # ---- output projection (p-major staging, dual-queue DMA)
                if b == BPC - 1:
                    for pair in range(2):
                        for j in range(2):
                            qt = 2 * pair + j
                            nc.tensor.matmul(
                                po2s[pair][:, j * DOUT:(j + 1) * DOUT],
                                lhsT=Unorm[:, C - 1, qt * P:(qt + 1) * P],
                                rhs=wo_sb[:, C - 1, :],
                                start=False, stop=True,
                                skip_group_check=True)
                    for pair in range(2):
                        sl = slice(2 * pair, 2 * pair + 2)
                        if pair == 0:
                            nc.scalar.copy(
                                out=ob[:, sl, :],
                                in_=po2s[pair][:].rearrange(
                                    "p (i n) -> p i n", i=2))
                            nc.sync.dma_start(
                                out[b].rearrange("p (i n) -> p i n", i=4)[:, sl, :],
                                ob[:, sl, :])
                        else:
                            nc.vector.tensor_copy(
                                out=ob[:, sl, :],
                                in_=po2s[pair][:].rearrange(
                                    "p (i n) -> p i n", i=2))
                            nc.scalar.dma_start(
                                out[b].rearrange("p (i n) -> p i n", i=4)[:, sl, :],
                                ob[:, sl, :])
                else:
                    for qt in range(NQ // P):
                        po = ps_acc.tile([P, DOUT], f32, tag="acc", name="po")
                        for c in range(C):
                            nc.tensor.matmul(
                                po[:], lhsT=Unorm[:, c, qt * P:(qt + 1) * P],
                                rhs=wo_sb[:, c, :],
                                start=(c == 0), stop=(c == C - 1))
                        # all copies on DVE: a copy on the ACT queue would
                        # delay the next batch's exps
                        nc.vector.tensor_copy(out=ob[:, qt, :], in_=po[:])
                    ov = out[b].rearrange("p (i n) -> p i n", i=4)
                    nc.sync.dma_start(ov[:, 0:2, :], ob[:, 0:2, :])
                    nc.sync.dma_start(ov[:, 2:4, :], ob[:, 2:4, :])

    nc.compile()
    return nc


def _pack_dr(x):
    """[512, N] -> [128, 2, C2, N]: partition p, block j holds contraction
    row cc*256 + 2p + j (the DoubleRow SBUF layout)."""
    n = x.shape[1]
    return np.ascontiguousarray(
        x.reshape(C2, P, 2, n).transpose(1, 2, 0, 3))


def kernel(q, k, v, mask, W_query, W_key, W_val, W_out):
    global LAST_RESULT
    _install_ntff_hook()
    import ml_dtypes
    from concourse.bass_utils import run_bass_kernel_spmd

    fp8_qk = os.environ.get("MHA_FP8QK", "1") == "1"
    key = ("nc", fp8_qk)
    if key not in _CACHE:
        _CACHE[key] = _build(fp8_qk)
    nc = _CACHE[key]

    bf = lambda a: np.asarray(a, np.float32).astype(ml_dtypes.bfloat16)
    f8 = lambda a: np.asarray(a, np.float32).astype(ml_dtypes.float8_e4m3)

    q = np.asarray(q, np.float32)
    k = np.asarray(k, np.float32)
    v = np.asarray(v, np.float32)
    wq_h = np.asarray(W_query, np.float32).transpose(1, 0, 2).reshape(DIN, HE)
    wk_h = np.asarray(W_key, np.float32).transpose(1, 0, 2).reshape(DIN, HE)
    wv_h = np.asarray(W_val, np.float32).transpose(1, 0, 2).reshape(DIN, HE)
    wo_h = np.asarray(W_out, np.float32).reshape(HE, DOUT)
    mn_full = (~np.asarray(mask, bool)).astype(np.float32)  # [B, NK]

    # p-major weight layouts
    wv_p = bf(wv_h.reshape(C, P, HE).transpose(1, 0, 2))      # [P, C, HE]
    wo_p = bf(wo_h.reshape(C, P, DOUT).transpose(1, 0, 2))    # [P, C, DOUT]

    if fp8_qk:
        wq_pk = _pack_dr(wq_h * SW)   # [P, 2, C2, HE] fp32
        wk_pk = _pack_dr(wk_h * SW)
    else:
        wq_c = bf(wq_h).reshape(C, P, HE)
        wk_c = bf(wk_h).reshape(C, P, HE)

    in_maps = []
    for i in range(NCORES):
        sl = slice(i * BPC, (i + 1) * BPC)
        qT_i = q[sl].transpose(0, 2, 1)   # [BPC, d, n]
        kT_i = k[sl].transpose(0, 2, 1)
        vT_i = v[sl].transpose(0, 2, 1)
        mn_i = mn_full[sl].reshape(BPC, T, P).transpose(0, 2, 1)  # [BPC,P,T]
        m = {
            "vT": bf(vT_i.reshape(BPC, C, P, NK).transpose(0, 2, 1, 3)),
            "wv": wv_p, "wo": wo_p,
            "mrep": bf(np.broadcast_to(
                mn_i[..., None], (BPC, P, T, NQ))),
            "mnot": np.ascontiguousarray(mn_i),
        }
        if fp8_qk:
            qp = np.stack([_pack_dr(qT_i[b]) for b in range(BPC)])
            kp = np.stack([_pack_dr(kT_i[b]) for b in range(BPC)])
            m["qTi"] = f8(qp)
            m["kTi"] = f8(kp)
            m["wqf"] = f8(np.concatenate([wq_pk, qp[0]], axis=3)
                          .transpose(2, 0, 1, 3))   # [C2, P, 2, HE+NQ]
            m["wkf"] = f8(np.concatenate([wk_pk, kp[0]], axis=3)
                          .transpose(2, 0, 1, 3))
        else:
            qc = bf(qT_i.reshape(BPC, C, P, NQ).transpose(0, 2, 1, 3))
            kc = bf(kT_i.reshape(BPC, C, P, NK).transpose(0, 2, 1, 3))
            m["qTi"] = qc
            m["kTi"] = kc
            m["wqf"] = np.ascontiguousarray(np.concatenate(
                [np.broadcast_to(wq_c[None], (1, C, P, HE))[0],
                 qc[0].transpose(1, 0, 2)], axis=2))
            m["wkf"] = np.ascontiguousarray(np.concatenate(
                [wk_c, kc[0].transpose(1, 0, 2)], axis=2))
        in_maps.append(m)

    res = run_bass_kernel_spmd(nc, in_maps, core_ids=list(range(NCORES)))
    LAST_RESULT = res
    # out is [BPC, P, 4, DOUT] p-major -> [BPC, NQ, DOUT]
    return np.concatenate(
        [np.asarray(r["out"], np.float32).reshape(BPC, P, NQ // P, DOUT)
         .transpose(0, 2, 1, 3).reshape(BPC, NQ, DOUT)
         for r in res.results], axis=0)
